# revision 3
# baseline (speedup 1.0000x reference)
"""Trainium2 Bass kernel for batched multi-head attention.

Problem: query/key/value [B=2, H=16, S=2048, D=64] fp32, per-(b,h) divisor
`inv_scale_factor` [B, H, 1, 1].  out = softmax(Q K^T / inv_scale) V.

Sharding: the 32 (b,h) heads are split across 8 NeuronCores, 4 heads per
core, fully data-parallel (no collectives).  Each core runs the same
program on its own 4-head slice.

Per-core algorithm (per head, Sq tiled into q-blocks of 1024):
  - Load Q, K naturally ([128 seq, 64 d] tiles), transpose on the PE
    against an identity matrix to get Q^T / K^T with d on partitions,
    rounding to float32r (12-bit mantissa, full-rate PE) on the
    PSUM->SBUF copy.
  - scores_T[kv, q] = K^T_tile.T @ Q^T (PE, f32r, kv on partitions).
  - P^T = exp(scores_T * (1/inv_scale)) on the ACT engine straight out of
    PSUM, with the runtime per-head 1/inv_scale folded into the
    activation's per-partition scale operand.  No max-subtraction is
    needed: |qk/s| <= ~13 so exp stays well inside fp32 range.
  - PV uses V augmented with a ones column ([kv, 65] stationary), so the
    softmax denominator (column 64) falls out of the same accumulating
    matmul that computes P^T.T-contracted V.
  - The [65, q] accumulator is copied to SBUF, transposed back on the PE,
    and each [128 q, 64 d] tile is scaled by 1/denominator (DVE
    reciprocal + per-partition tensor_scalar) before the store.
"""

import numpy as np

import concourse.bass as bass
import concourse.tile as tile
from concourse import bacc, mybir
from concourse.bass_utils import run_bass_kernel_spmd
from concourse.masks import make_identity

F32 = mybir.dt.float32
F32R = mybir.dt.float32r
EXP = mybir.ActivationFunctionType.Exp

B, H, SQ, SKV, D = 2, 16, 2048, 2048, 64
N_CORES = 8
HEADS_PER_CORE = (B * H) // N_CORES  # 4


def build_attention(nh=HEADS_PER_CORE, sq=SQ, skv=SKV, d=D, qblock=1024,
                    num_devices=N_CORES, enable_asserts=False):
    """Build the per-core Bass program. Returns the compiled Bacc module."""
    assert d == 64
    assert sq % 128 == 0 and skv % 128 == 0
    qblock = min(qblock, sq)
    assert sq % qblock == 0
    nchunk = min(512, qblock)          # matmul moving free-dim chunk
    assert qblock % nchunk == 0
    ntq = sq // 128                    # q tiles per head
    nkv = skv // 128                   # kv tiles per head
    nqb = sq // qblock                 # q blocks per head
    ntq_b = qblock // 128              # q tiles per q block

    nc = bacc.Bacc("TRN2", target_bir_lowering=False, debug=False,
                   enable_asserts=enable_asserts, num_devices=num_devices)

    q_dram = nc.dram_tensor("query", [nh, sq, d], F32, kind="ExternalInput").ap()
    k_dram = nc.dram_tensor("key", [nh, skv, d], F32, kind="ExternalInput").ap()
    v_dram = nc.dram_tensor("value", [nh, skv, d], F32, kind="ExternalInput").ap()
    inv_dram = nc.dram_tensor("inv_scale", [1, nh], F32, kind="ExternalInput").ap()
    o_dram = nc.dram_tensor("out", [nh, sq, d], F32, kind="ExternalOutput").ap()

    with tile.TileContext(nc) as tc:
        _attention_body(tc, o_dram, q_dram, k_dram, v_dram, inv_dram,
                        nh, sq, skv, d, qblock, nchunk, ntq, nkv, nqb, ntq_b)

    nc.compile()
    return nc


def _attention_body(tc, o_dram, q_dram, k_dram, v_dram, inv_dram,
                    nh, sq, skv, d, qblock, nchunk, ntq, nkv, nqb, ntq_b):
    nc = tc.nc
    from contextlib import ExitStack
    with ExitStack() as ctx:
        const = ctx.enter_context(tc.tile_pool(name="const", bufs=1))
        qnatp = ctx.enter_context(tc.tile_pool(name="qnat", bufs=2))
        knatp = ctx.enter_context(tc.tile_pool(name="knat", bufs=2))
        vnatp = ctx.enter_context(tc.tile_pool(name="vnat", bufs=2))
        qtp = ctx.enter_context(tc.tile_pool(name="qt", bufs=2))
        ktp = ctx.enter_context(tc.tile_pool(name="kt", bufs=2))
        vaugp = ctx.enter_context(tc.tile_pool(name="vaug", bufs=2))
        ptp = ctx.enter_context(tc.tile_pool(name="pt", bufs=3))
        osbp = ctx.enter_context(tc.tile_pool(name="osb", bufs=2))
        finp = ctx.enter_context(tc.tile_pool(name="fin", bufs=2))
        recp = ctx.enter_context(tc.tile_pool(name="rec", bufs=4))
        scp = ctx.enter_context(tc.tile_pool(name="scps", bufs=2, space="PSUM"))
        outp = ctx.enter_context(tc.tile_pool(name="outps", bufs=1, space="PSUM"))
        tpp = ctx.enter_context(tc.tile_pool(name="tpps", bufs=2, space="PSUM"))

        # --- constants: identity, per-head 1/inv_scale broadcast to [128, nh]
        ident = const.tile([128, 128], F32)
        make_identity(nc, ident[:])
        inv_sb = const.tile([1, nh], F32)
        nc.sync.dma_start(inv_sb[:], inv_dram[:])
        recip_sb = const.tile([1, nh], F32)
        nc.vector.reciprocal(recip_sb[:], inv_sb[:])
        ones_row = const.tile([1, 128], F32)
        nc.vector.memset(ones_row[:], 1.0)
        bps = tpp.tile([128, 128], F32, tag="tp")
        nc.tensor.matmul(bps[0:128, 0:nh], ones_row[0:1, 0:128],
                         recip_sb[0:1, 0:nh], start=True, stop=True)
        scale_all = const.tile([128, nh], F32)
        nc.vector.tensor_copy(scale_all[:], bps[0:128, 0:nh])

        for h in range(nh):
            # ---------------- input staging ----------------
            qnat = qnatp.tile([128, ntq * d], F32, tag="qnat")
            nc.sync.dma_start(
                qnat[:].rearrange("p (t e) -> p t e", e=d),
                q_dram[h].rearrange("(t p) e -> p t e", p=128))
            knat = knatp.tile([128, nkv * d], F32, tag="knat")
            nc.sync.dma_start(
                knat[:].rearrange("p (t e) -> p t e", e=d),
                k_dram[h].rearrange("(t p) e -> p t e", p=128))
            # V staged with a ones column -> [kv, d+1] stationaries (f32r)
            vnat = vnatp.tile([128, nkv * (d + 1)], F32, tag="vnat")
            nc.vector.memset(vnat[:], 1.0)
            nc.sync.dma_start(
                vnat[:].rearrange("p (t e) -> p t e", e=d + 1)[:, :, 0:d],
                v_dram[h].rearrange("(t p) e -> p t e", p=128))
            vaug = vaugp.tile([128, nkv * (d + 1)], F32R, tag="vaug")
            nc.vector.tensor_copy(vaug[:], vnat[:])

            # Q^T, K^T via PE transposes (f32r rounded on the PSUM->SBUF copy)
            qt = qtp.tile([64, sq], F32R, tag="qt")
            for t in range(ntq):
                psq = tpp.tile([128, 128], F32, tag="tp")
                nc.tensor.transpose(psq[0:64, 0:128],
                                    qnat[:, t * d:(t + 1) * d],
                                    ident[0:128, 0:128])
                nc.vector.tensor_copy(qt[0:64, t * 128:(t + 1) * 128],
                                      psq[0:64, 0:128])
            kt = ktp.tile([64, skv], F32R, tag="kt")
            for t in range(nkv):
                psk = tpp.tile([128, 128], F32, tag="tp")
                nc.tensor.transpose(psk[0:64, 0:128],
                                    knat[:, t * d:(t + 1) * d],
                                    ident[0:128, 0:128])
                nc.vector.tensor_copy(kt[0:64, t * 128:(t + 1) * 128],
                                      psk[0:64, 0:128])

            scale_h = scale_all[:, h:h + 1]

            # ---------------- main loop ----------------
            for qb in range(nqb):
                q0 = qb * qblock
                out_ps = outp.tile([65, qblock], F32, tag="out")
                for kvt in range(nkv):
                    sc = scp.tile([128, qblock], F32, tag="sc")
                    for c in range(qblock // nchunk):
                        nc.tensor.matmul(
                            sc[:, c * nchunk:(c + 1) * nchunk],
                            kt[0:64, kvt * 128:(kvt + 1) * 128],
                            qt[0:64, q0 + c * nchunk:q0 + (c + 1) * nchunk],
                            start=True, stop=True)
                    pt = ptp.tile([128, qblock], F32R, tag="pt")
                    nc.scalar.activation(pt[:], sc[:], EXP, scale=scale_h)
                    for c in range(qblock // nchunk):
                        nc.tensor.matmul(
                            out_ps[0:65, c * nchunk:(c + 1) * nchunk],
                            vaug[:, kvt * (d + 1):(kvt + 1) * (d + 1)],
                            pt[:, c * nchunk:(c + 1) * nchunk],
                            start=(kvt == 0), stop=(kvt == nkv - 1))

                # ---------------- epilogue for this q block ----------------
                osb = osbp.tile([65, qblock], F32, tag="osb")
                nc.vector.tensor_copy(osb[:], out_ps[0:65, :])
                fin = finp.tile([128, ntq_b * d], F32, tag="fin")
                for st in range(ntq_b):
                    pso = tpp.tile([128, 128], F32, tag="tp")
                    nc.tensor.transpose(pso[0:128, 0:65],
                                        osb[0:65, st * 128:(st + 1) * 128],
                                        ident[0:65, 0:65])
                    rec = recp.tile([128, 1], F32, tag="rec")
                    nc.vector.reciprocal(rec[:], pso[:, 64:65])
                    nc.vector.tensor_scalar_mul(fin[:, st * d:(st + 1) * d],
                                                pso[:, 0:d], rec[:])
                nc.sync.dma_start(
                    o_dram[h].rearrange("(t p) e -> p t e", p=128)[
                        :, qb * ntq_b:(qb + 1) * ntq_b, :],
                    fin[:].rearrange("p (t e) -> p t e", e=d))


_NC_CACHE = {}


def _get_program():
    key = "full"
    if key not in _NC_CACHE:
        _NC_CACHE[key] = build_attention()
    return _NC_CACHE[key]


def kernel(query, key, value, inv_scale_factor):
    """Full-input entry point: shard over 8 cores, run, gather."""
    nc = _get_program()
    q = np.ascontiguousarray(query, dtype=np.float32).reshape(B * H, SQ, D)
    k = np.ascontiguousarray(key, dtype=np.float32).reshape(B * H, SKV, D)
    v = np.ascontiguousarray(value, dtype=np.float32).reshape(B * H, SKV, D)
    inv = np.ascontiguousarray(inv_scale_factor, dtype=np.float32).reshape(B * H)

    hpc = HEADS_PER_CORE
    in_maps = []
    for c in range(N_CORES):
        s = slice(c * hpc, (c + 1) * hpc)
        in_maps.append({
            "query": q[s],
            "key": k[s],
            "value": v[s],
            "inv_scale": inv[s].reshape(1, hpc),
        })
    res = run_bass_kernel_spmd(nc, in_maps, core_ids=list(range(N_CORES)))
    out = np.concatenate([res.results[c]["out"] for c in range(N_CORES)], axis=0)
    return out.reshape(B, H, SQ, D)


# revision 6
# speedup vs baseline: 1.1167x; 1.1167x over previous
"""Trainium2 Bass kernel for batched multi-head attention.

Problem: query/key/value [B=2, H=16, S=2048, D=64] fp32, per-(b,h) divisor
`inv_scale_factor` [B, H, 1, 1].  out = softmax(Q K^T / inv_scale) V.

Sharding: the 32 (b,h) heads are split across 8 NeuronCores, 4 heads per
core, fully data-parallel (no collectives).  Each core runs the same
program on its own 4-head slice.

Per-core algorithm (per head, Sq tiled into q-blocks of 1024):
  - Load Q, K, V naturally ([128 seq, 64 d] tiles), cast to fp16 on DVE.
  - Transpose Q and K tiles on the PE as *regular* fp16 matmuls against an
    fp16 identity (out = tile.T @ I in fp32 PSUM, exact), giving Q^T / K^T
    with d on partitions; the PSUM->SBUF copy casts back to fp16 (exact).
  - scores_T[kv, q] = K^T_tile.T @ Q^T on the PE (fp16 in, fp32 PSUM).
  - P^T = exp(scores_T * (1/inv_scale) - ln 16) on the ACT engine straight
    out of PSUM with fp16 output.  The runtime per-head 1/inv_scale is a
    per-partition scale operand; the -ln 16 bias keeps exp below fp16 max
    (|qk|/s <= ~13 -> exp*2^-4 <= ~2e4) and cancels in the normalization.
    No max-subtraction pass is needed.
  - PV uses V augmented with a ones column ([kv, 65] fp16 stationary), so
    the softmax denominator (row 64) falls out of the same accumulating
    matmul chain that contracts P^T with V.
  - The [65, q] fp32 accumulator is copied to SBUF, transposed back on the
    PE (fp32 transpose mode), and each [128 q, 64 d] tile is scaled by
    1/denominator (DVE reciprocal + per-partition tensor_scalar).
"""

import numpy as np

import concourse.bass as bass
import concourse.tile as tile
from concourse import bacc, mybir
from concourse.bass_utils import run_bass_kernel_spmd
from concourse.masks import make_identity

F32 = mybir.dt.float32
F16 = mybir.dt.float16
EXP = mybir.ActivationFunctionType.Exp
LN16 = float(np.log(16.0))

B, H, SQ, SKV, D = 2, 16, 2048, 2048, 64
N_CORES = 8
HEADS_PER_CORE = (B * H) // N_CORES  # 4


def build_attention(nh=HEADS_PER_CORE, sq=SQ, skv=SKV, d=D, qblock=1024,
                    num_devices=N_CORES, enable_asserts=False):
    """Build the per-core Bass program. Returns the compiled Bacc module."""
    assert d == 64
    assert sq % 128 == 0 and skv % 128 == 0
    qblock = min(qblock, sq)
    assert sq % qblock == 0
    nchunk = min(512, qblock)          # matmul moving free-dim chunk
    assert qblock % nchunk == 0
    ntq = sq // 128                    # q tiles per head
    nkv = skv // 128                   # kv tiles per head
    nqb = sq // qblock                 # q blocks per head
    ntq_b = qblock // 128              # q tiles per q block

    nc = bacc.Bacc("TRN2", target_bir_lowering=False, debug=False,
                   enable_asserts=enable_asserts, num_devices=num_devices)

    q_dram = nc.dram_tensor("query", [nh, sq, d], F32, kind="ExternalInput").ap()
    k_dram = nc.dram_tensor("key", [nh, skv, d], F32, kind="ExternalInput").ap()
    v_dram = nc.dram_tensor("value", [nh, skv, d], F32, kind="ExternalInput").ap()
    inv_dram = nc.dram_tensor("inv_scale", [1, nh], F32, kind="ExternalInput").ap()
    o_dram = nc.dram_tensor("out", [nh, sq, d], F32, kind="ExternalOutput").ap()

    with tile.TileContext(nc) as tc:
        _attention_body(tc, o_dram, q_dram, k_dram, v_dram, inv_dram,
                        nh, sq, skv, d, qblock, nchunk, ntq, nkv, nqb, ntq_b)

    nc.compile()
    return nc


def _attention_body(tc, o_dram, q_dram, k_dram, v_dram, inv_dram,
                    nh, sq, skv, d, qblock, nchunk, ntq, nkv, nqb, ntq_b):
    nc = tc.nc
    from contextlib import ExitStack
    with ExitStack() as ctx:
        const = ctx.enter_context(tc.tile_pool(name="const", bufs=1))
        qnatp = ctx.enter_context(tc.tile_pool(name="qnat", bufs=2))
        knatp = ctx.enter_context(tc.tile_pool(name="knat", bufs=2))
        vnatp = ctx.enter_context(tc.tile_pool(name="vnat", bufs=2))
        qhp = ctx.enter_context(tc.tile_pool(name="qh", bufs=2))
        khp = ctx.enter_context(tc.tile_pool(name="kh", bufs=2))
        qtp = ctx.enter_context(tc.tile_pool(name="qt", bufs=2))
        ktp = ctx.enter_context(tc.tile_pool(name="kt", bufs=2))
        vaugp = ctx.enter_context(tc.tile_pool(name="vaug", bufs=2))
        ptp = ctx.enter_context(tc.tile_pool(name="pt", bufs=3))
        osbp = ctx.enter_context(tc.tile_pool(name="osb", bufs=2))
        finp = ctx.enter_context(tc.tile_pool(name="fin", bufs=2))
        recp = ctx.enter_context(tc.tile_pool(name="rec", bufs=4))
        scp = ctx.enter_context(tc.tile_pool(name="scps", bufs=2, space="PSUM"))
        outp = ctx.enter_context(tc.tile_pool(name="outps", bufs=1, space="PSUM"))
        tpp = ctx.enter_context(tc.tile_pool(name="tpps", bufs=2, space="PSUM"))

        # --- constants: identities, per-head 1/inv_scale broadcast [128, nh]
        ident = const.tile([128, 128], F32)
        make_identity(nc, ident[:])
        ident_h = const.tile([128, 128], F16)
        nc.vector.tensor_copy(ident_h[:], ident[:])
        inv_sb = const.tile([1, nh], F32)
        nc.sync.dma_start(inv_sb[:], inv_dram[:])
        recip_sb = const.tile([1, nh], F32)
        nc.vector.reciprocal(recip_sb[:], inv_sb[:])
        ones_row = const.tile([1, 128], F32)
        nc.vector.memset(ones_row[:], 1.0)
        bias_col = const.tile([128, 1], F32)
        nc.vector.memset(bias_col[:], -LN16)
        bps = tpp.tile([128, 128], F32, tag="tp")
        nc.tensor.matmul(bps[0:128, 0:nh], ones_row[0:1, 0:128],
                         recip_sb[0:1, 0:nh], start=True, stop=True)
        scale_all = const.tile([128, nh], F32)
        nc.vector.tensor_copy(scale_all[:], bps[0:128, 0:nh])

        for h in range(nh):
            # ---------------- input staging ----------------
            qnat = qnatp.tile([128, ntq * d], F32, tag="qnat")
            nc.sync.dma_start(
                qnat[:].rearrange("p (t e) -> p t e", e=d),
                q_dram[h].rearrange("(t p) e -> p t e", p=128))
            knat = knatp.tile([128, nkv * d], F32, tag="knat")
            nc.sync.dma_start(
                knat[:].rearrange("p (t e) -> p t e", e=d),
                k_dram[h].rearrange("(t p) e -> p t e", p=128))
            qh16 = qhp.tile([128, ntq * d], F16, tag="qh")
            nc.vector.tensor_copy(qh16[:], qnat[:])
            kh16 = khp.tile([128, nkv * d], F16, tag="kh")
            nc.vector.tensor_copy(kh16[:], knat[:])

            # V staged with a ones column -> [kv, d+1] fp16 stationaries
            vnat = vnatp.tile([128, nkv * (d + 1)], F32, tag="vnat")
            nc.vector.memset(vnat[:], 1.0)
            nc.sync.dma_start(
                vnat[:].rearrange("p (t e) -> p t e", e=d + 1)[:, :, 0:d],
                v_dram[h].rearrange("(t p) e -> p t e", p=128))
            vaug = vaugp.tile([128, nkv * (d + 1)], F16, tag="vaug")
            nc.vector.tensor_copy(vaug[:], vnat[:])

            # Q^T, K^T via regular fp16 matmuls against identity (exact)
            qt = qtp.tile([64, sq], F16, tag="qt")
            for t in range(ntq):
                psq = tpp.tile([128, 128], F32, tag="tp")
                nc.tensor.matmul(psq[0:64, 0:128],
                                 qh16[:, t * d:(t + 1) * d],
                                 ident_h[0:128, 0:128], start=True, stop=True)
                nc.vector.tensor_copy(qt[0:64, t * 128:(t + 1) * 128],
                                      psq[0:64, 0:128])
            kt = ktp.tile([64, skv], F16, tag="kt")
            for t in range(nkv):
                psk = tpp.tile([128, 128], F32, tag="tp")
                nc.tensor.matmul(psk[0:64, 0:128],
                                 kh16[:, t * d:(t + 1) * d],
                                 ident_h[0:128, 0:128], start=True, stop=True)
                nc.vector.tensor_copy(kt[0:64, t * 128:(t + 1) * 128],
                                      psk[0:64, 0:128])

            scale_h = scale_all[:, h:h + 1]

            # ---------------- main loop ----------------
            for qb in range(nqb):
                q0 = qb * qblock
                out_ps = outp.tile([65, qblock], F32, tag="out")
                for kvt in range(nkv):
                    sc = scp.tile([128, qblock], F32, tag="sc")
                    for c in range(qblock // nchunk):
                        nc.tensor.matmul(
                            sc[:, c * nchunk:(c + 1) * nchunk],
                            kt[0:64, kvt * 128:(kvt + 1) * 128],
                            qt[0:64, q0 + c * nchunk:q0 + (c + 1) * nchunk],
                            start=True, stop=True)
                    pt = ptp.tile([128, qblock], F16, tag="pt")
                    nc.scalar.activation(pt[:], sc[:], EXP,
                                         bias=bias_col[:], scale=scale_h)
                    for c in range(qblock // nchunk):
                        nc.tensor.matmul(
                            out_ps[0:65, c * nchunk:(c + 1) * nchunk],
                            vaug[:, kvt * (d + 1):(kvt + 1) * (d + 1)],
                            pt[:, c * nchunk:(c + 1) * nchunk],
                            start=(kvt == 0), stop=(kvt == nkv - 1))

                # ---------------- epilogue for this q block ----------------
                osb = osbp.tile([65, qblock], F32, tag="osb")
                nc.vector.tensor_copy(osb[:], out_ps[0:65, :])
                fin = finp.tile([128, ntq_b * d], F32, tag="fin")
                for st in range(ntq_b):
                    pso = tpp.tile([128, 128], F32, tag="tp")
                    nc.tensor.transpose(pso[0:128, 0:65],
                                        osb[0:65, st * 128:(st + 1) * 128],
                                        ident[0:65, 0:65])
                    rec = recp.tile([128, 1], F32, tag="rec")
                    nc.vector.reciprocal(rec[:], pso[:, 64:65])
                    nc.vector.tensor_scalar_mul(fin[:, st * d:(st + 1) * d],
                                                pso[:, 0:d], rec[:])
                nc.sync.dma_start(
                    o_dram[h].rearrange("(t p) e -> p t e", p=128)[
                        :, qb * ntq_b:(qb + 1) * ntq_b, :],
                    fin[:].rearrange("p (t e) -> p t e", e=d))


_NC_CACHE = {}


def _get_program():
    key = "full"
    if key not in _NC_CACHE:
        _NC_CACHE[key] = build_attention()
    return _NC_CACHE[key]


def kernel(query, key, value, inv_scale_factor):
    """Full-input entry point: shard over 8 cores, run, gather."""
    nc = _get_program()
    q = np.ascontiguousarray(query, dtype=np.float32).reshape(B * H, SQ, D)
    k = np.ascontiguousarray(key, dtype=np.float32).reshape(B * H, SKV, D)
    v = np.ascontiguousarray(value, dtype=np.float32).reshape(B * H, SKV, D)
    inv = np.ascontiguousarray(inv_scale_factor, dtype=np.float32).reshape(B * H)

    hpc = HEADS_PER_CORE
    in_maps = []
    for c in range(N_CORES):
        s = slice(c * hpc, (c + 1) * hpc)
        in_maps.append({
            "query": q[s],
            "key": k[s],
            "value": v[s],
            "inv_scale": inv[s].reshape(1, hpc),
        })
    res = run_bass_kernel_spmd(nc, in_maps, core_ids=list(range(N_CORES)))
    out = np.concatenate([res.results[c]["out"] for c in range(N_CORES)], axis=0)
    return out.reshape(B, H, SQ, D)
